# revision 1
# baseline (speedup 1.0000x reference)
"""Multi-Head Latent Attention (naive MLA) on 8 Trainium2 NeuronCores.

Sharding: data-parallel over batch (2) x causal-balanced sequence-parallel
over queries (4-way interleave): core c handles batch b = c//4, query group
g = c%4 (global query rows g, g+4, g+8, ...).  Every core runs the identical
SPMD program; only the data (x shards, wedge-mask matrices) differs.
No collectives: each core produces the full output rows for its queries.

All matmuls contract over the SBUF partition dim, so everything is kept
"transposed" (feature-major) end to end and no on-device transposes are
needed:
  latentT = Wdkv^T @ x^T                  [128, 2048]  f32r
  qT      = Wq^T @ xq^T                   [1024, 512]  bf16
  kT      = Wuk^T @ latentT               [1024, 2048] bf16
  v_aug   = [latent @ Wuv | ones] per key tile          bf16
  scoresT: per head, psum[128 keys, q] = matmul(lhsT=kT tile, rhs=qT tile);
           causal wedge added by a rank-32 mask matmul; exp on ScalarE with
           the 1/sqrt(hd) folded into the activation scale; softmax denom
           comes for free as row 64 of the ctx matmul (ones column of v_aug)
  ctxT_h  = v_aug^T @ expT                [65, q] psum accum over key tiles
  out     = matmul(lhsT=ctxT tiles, rhs=Wo) + bo  -> DRAM [512, 1024]
"""

import numpy as np

B, S, D, L, H = 2, 2048, 1024, 128, 16
HD = D // H        # 64
AUG = HD + 1       # 65 (v dims + ones column for softmax denominator)
NCORES = 8
GQ = S // 4        # 512 queries per core
QT = 256           # queries per q-tile
NT = GQ // QT      # 2 q-tiles
KT = 128           # keys per key tile
NKT = S // KT      # 16
NEG = -640.0       # additive mask pre-exp-scale (x 1/8 -> -80)

_cache = {}


def _worklist(offset):
    """Per q-tile t: list of (u, cs, wedge), identical across cores.

    Query column c of tile t = global row 4*(QT*t+c)+g, position +offset.
    cs (first computed column of the strip) uses the worst core (g=3) so
    strip shapes are core-independent; the wedge matrix (data) carries g.
    """
    work = []
    for t in range(NT):
        items = []
        for u in range(NKT):
            lo = KT * u
            min_qpos = 4 * (QT * t) + 0 + offset
            max_qpos = 4 * (QT * t + QT - 1) + 3 + offset
            if lo + KT - 1 <= min_qpos:
                items.append((u, 0, False))      # fully allowed
            elif lo > max_qpos:
                continue                         # fully masked: skip
            else:
                cs = max(0, -((-(lo - 3 - offset)) // 4) - QT * t)
                assert 0 <= cs < QT
                items.append((u, cs, True))
        assert items and items[0][1] == 0, "first strip must cover col 0"
        work.append(items)
    return work


def _wedge_matrix(g, offset, work):
    """[32, 128] f32: T[m, kj] = NEG where key kj is masked at strip col m.

    Strip col c' (from cs): masked iff kj > 4*c' + r0,
    r0 = 4*(QT*t+cs)+g+offset-lo.  r0 must be tile-independent (asserted)
    so a single matrix serves every partial tile of this core.
    """
    r0s = set()
    for t, items in enumerate(work):
        for (u, cs, wedge) in items:
            if wedge:
                r0s.add(4 * (QT * t + cs) + g + offset - KT * u)
    if not r0s:
        r0s = {g}
    assert len(r0s) == 1, f"non-uniform wedge r0 {r0s} (offset={offset})"
    r0 = r0s.pop()
    assert 0 <= r0 <= 127, r0
    T = np.zeros((32, 128), np.float32)
    for m in range(32):
        T[m, :] = np.where(np.arange(128) > 4 * m + r0, NEG, 0.0)
    return T


def _blocks_of(items):
    """Pack strips into single-bank psum bins of [128, 512] (matmul psum
    outputs can't cross banks).  Returns [(list[((u,cs,wedge), off)], fill)]."""
    bins = []
    cur, w = [], 0
    for it in items:
        sw = QT - it[1]
        if w + sw > 512:
            bins.append((cur, w))
            cur, w = [], 0
        cur.append((it, w))
        w += sw
    if cur:
        bins.append((cur, w))
    return bins


def _build(offset):
    import concourse.bacc as bacc
    import concourse.tile as tile
    import concourse.mybir as mybir
    from contextlib import ExitStack

    f32r = mybir.dt.float32r
    bf16 = mybir.dt.bfloat16
    f32 = mybir.dt.float32
    AF = mybir.ActivationFunctionType
    ALU = mybir.AluOpType

    work = _worklist(offset)

    nc = bacc.Bacc("TRN2", target_bir_lowering=False, debug=False,
                   num_devices=NCORES)
    xT = nc.dram_tensor("xT", [D, S], bf16, kind="ExternalInput").ap()
    xqT = nc.dram_tensor("xqT", [D, GQ], f32r, kind="ExternalInput").ap()
    Wq = nc.dram_tensor("Wq", [D, D], f32r, kind="ExternalInput").ap()
    Wdkv = nc.dram_tensor("Wdkv", [D, L], bf16, kind="ExternalInput").ap()
    Wukv = nc.dram_tensor("Wukv", [L, 2 * D], f32r, kind="ExternalInput").ap()
    Wo = nc.dram_tensor("Wo", [D, D], f32r, kind="ExternalInput").ap()
    bo = nc.dram_tensor("bo", [1, D], f32r, kind="ExternalInput").ap()
    Twedge = nc.dram_tensor("Twedge", [32, 128], bf16,
                            kind="ExternalInput").ap()
    I32 = nc.dram_tensor("I32", [32, 32], bf16, kind="ExternalInput").ap()
    Ones = nc.dram_tensor("Ones", [1, 130], f32r, kind="ExternalInput").ap()
    out = nc.dram_tensor("out", [GQ, D], f32, kind="ExternalOutput").ap()

    with tile.TileContext(nc) as tc, ExitStack() as ctx:
        const = ctx.enter_context(tc.tile_pool(name="const", bufs=1, side="right"))
        pp = ctx.enter_context(tc.tile_pool(name="pp", bufs=1, side="right"))
        precs = ctx.enter_context(tc.tile_pool(name="precs", bufs=2, side="right"))
        psc = ctx.enter_context(tc.tile_pool(name="psc", bufs=4, space="PSUM", side="left"))
        pctx = ctx.enter_context(
            tc.tile_pool(name="pctx", bufs=2, space="PSUM", side="right"))
        pout = ctx.enter_context(
            tc.tile_pool(name="pout", bufs=2, space="PSUM", side="right"))
        sexp = ctx.enter_context(tc.tile_pool(name="sexp", bufs=12, side="left"))
        sout = ctx.enter_context(tc.tile_pool(name="sout", bufs=3, side="left"))
        pb1 = ctx.enter_context(tc.tile_pool(name="pb1", bufs=1,
                                              side="left"))
        pal_cm = tc.tile_pool(name="pal", bufs=1, side="left")
        pal = pal_cm.__enter__()
        par_cm = tc.tile_pool(name="par", bufs=1, side="right")
        pAr = par_cm.__enter__()

        ones_sb = const.tile([1, 128], f32r)
        nc.sync.dma_start(ones_sb[:], Ones[:, 0:128])

        # ---------- phase 1: loads; latentT; qT ----------
        xT_sb = pal.tile([128, D // 128, S], bf16, tag="xT")
        Wdkv_sb = pal.tile([128, D // 128, L], bf16, tag="Wdkv")
        xqT_sb = pAr.tile([128, D // 128, GQ], f32r, tag="xqT")
        Wq_sb = pal.tile([128, D // 128, D], f32r, tag="Wq")
        for k in range(D // 128):
            nc.sync.dma_start(Wdkv_sb[:, k, :],
                              Wdkv[128 * k:128 * (k + 1), :])
            nc.sync.dma_start(xT_sb[:, k, :], xT[128 * k:128 * (k + 1), :])
        for k in range(D // 128):
            nc.sync.dma_start(xqT_sb[:, k, :], xqT[128 * k:128 * (k + 1), :])
            nc.sync.dma_start(Wq_sb[:, k, :], Wq[128 * k:128 * (k + 1), :])
        Wukv_sb = pb1.tile([128, 2 * D], f32r, tag="Wukv")
        nc.sync.dma_start(Wukv_sb[:], Wukv[:])

        latT_sb = pb1.tile([128, S], f32r, tag="latT")
        for n in range(S // 512):
            ps = psc.tile([128, 512], f32, tag="sc", name=f"lat_ps{n}")
            for k in range(D // 128):
                nc.tensor.matmul(ps[:], Wdkv_sb[:, k, :],
                                 xT_sb[:, k, 512 * n:512 * (n + 1)],
                                 start=(k == 0), stop=(k == D // 128 - 1))
            nc.scalar.copy(latT_sb[:, 512 * n:512 * (n + 1)], ps[:])

        qT_sb = pp.tile([128, H // 2, GQ], bf16, tag="qT")
        for m in range(H // 2):
            ps = psc.tile([128, GQ], f32, tag="sc", name=f"q_ps{m}")
            for k in range(D // 128):
                nc.tensor.matmul(ps[:], Wq_sb[:, k, 128 * m:128 * (m + 1)],
                                 xqT_sb[:, k, :],
                                 start=(k == 0), stop=(k == D // 128 - 1))
            if m % 2 == 0:
                nc.vector.tensor_copy(qT_sb[:, m, :], ps[:])
            else:
                nc.scalar.copy(qT_sb[:, m, :], ps[:])

        # ---------- phase 2: kT; v_aug ----------
        pal_cm.__exit__(None, None, None)
        par_cm.__exit__(None, None, None)
        pb = ctx.enter_context(tc.tile_pool(name="pb", bufs=1, side="left"))
        kT_sb = pb.tile([128, H // 2, S], bf16, tag="kT")
        for m in range(H // 2):
            for n in range(S // 512):
                ps = psc.tile([128, 512], f32, tag="sc",
                              name=f"k_ps{m}_{n}")
                nc.tensor.matmul(ps[:], Wukv_sb[:, 128 * m:128 * (m + 1)],
                                 latT_sb[:, 512 * n:512 * (n + 1)],
                                 start=True, stop=True)
                if (m + n) % 2 == 0:
                    nc.vector.tensor_copy(
                        kT_sb[:, m, 512 * n:512 * (n + 1)], ps[:])
                else:
                    nc.scalar.copy(kT_sb[:, m, 512 * n:512 * (n + 1)],
                                   ps[:])

        va_sb = pb.tile([128, NKT, H * AUG], bf16, tag="va")
        for u in range(NKT):
            for half in range(2):      # heads 0-7 / 8-15
                ps = psc.tile([128, 512], f32, tag="sc",
                              name=f"v_ps{u}_{half}")
                nc.tensor.matmul(
                    ps[:], latT_sb[:, 128 * u:128 * (u + 1)],
                    Wukv_sb[:, D + 512 * half:D + 512 * (half + 1)],
                    start=True, stop=True)
                dst = va_sb[:, u, AUG * 8 * half:AUG * 8 * (half + 1)]
                if (u + half) % 2 == 0:
                    nc.vector.tensor_copy(
                        dst.rearrange("p (h e) -> p h e", e=AUG)[:, :, 0:HD],
                        ps[:].rearrange("p (h e) -> p h e", e=HD))
                else:
                    nc.scalar.copy(
                        dst.rearrange("p (h e) -> p h e", e=AUG)[:, :, 0:HD],
                        ps[:].rearrange("p (h e) -> p h e", e=HD))
        nc.any.memset(
            va_sb[:].rearrange("p u (h e) -> p u h e", e=AUG)[:, :, :, HD],
            1.0)

        # ---------- phase 3: attention (software-pipelined) ----------
        pc = ctx.enter_context(tc.tile_pool(name="pc", bufs=1, side="right"))
        ctxT_sb = pc.tile([128, H // 2, GQ], f32r, tag="ctxT")
        bo_sb = pc.tile([1, D], f32r, tag="bo")
        nc.sync.dma_start(bo_sb[:], bo[:])
        tw_sb = pc.tile([32, 128], bf16, tag="tw")
        nc.sync.dma_start(tw_sb[:], Twedge[:])
        i32_sb = pc.tile([32, 32], bf16, tag="i32")
        nc.sync.dma_start(i32_sb[:], I32[:])
        pd = ctx.enter_context(tc.tile_pool(name="pd", bufs=1, side="left"))
        Wo_sb = pd.tile([128, D // 128, D], f32r, tag="Wo")
        nc.sync.dma_start(Wo_sb[:], Wo.rearrange("(a p) n -> p a n", p=128))

        # flat work list: one entry per (head pair, q-tile, psum bin); the
        # PE stream is emitted one bin ahead of exp/ctx so the in-order PE
        # never waits on ScalarE's exp of the current bin.
        tbins = [_blocks_of(work[t]) for t in range(NT)]
        flat = []
        for hp in range(H // 2):
            for t in range(NT):
                for bi, (items, fill) in enumerate(tbins[t]):
                    flat.append((hp, t, bi, items, fill))
        state = {}   # (hp, t) -> [cps pair, done count, n_items]

        def emit_scores(idx):
            hp, t, bi, items, fill = flat[idx]
            sps = [psc.tile([128, 512], f32, tag="sc", name=f"sps{idx}_{p}")
                   for p in range(2)]
            for par in range(2):
                p0 = 64 * par
                for (u, cs, wedge), o in items:
                    sw = QT - cs
                    nc.tensor.matmul(
                        sps[par][:, o:o + sw],
                        kT_sb[p0:p0 + 64, hp, KT * u:KT * (u + 1)],
                        qT_sb[p0:p0 + 64, hp, QT * t + cs:QT * (t + 1)],
                        start=True, stop=not wedge)
                    if wedge:
                        wn = min(32, sw)
                        nc.tensor.matmul(sps[par][:, o:o + wn], tw_sb[:],
                                         i32_sb[:, 0:wn],
                                         start=False, stop=True)
            return sps

        def emit_exp(idx, sps):
            hp, t, bi, items, fill = flat[idx]
            exps = [sexp.tile([128, 512], bf16, tag="exp",
                              name=f"exp{idx}_{p}") for p in range(2)]
            for par in range(2):
                nc.scalar.activation(exps[par][:, 0:fill],
                                     sps[par][:, 0:fill],
                                     AF.Exp, scale=0.125)
            return exps

        def emit_ctx(idx, exps):
            hp, t, bi, items, fill = flat[idx]
            if (hp, t) not in state:
                cps = [pctx.tile([AUG, QT], f32, tag="ctx",
                                 name=f"cps{hp}_{t}_{p}") for p in range(2)]
                state[(hp, t)] = [cps, 0,
                                  sum(len(b[0]) for b in tbins[t])]
            cps, done, n_items = state[(hp, t)]
            for (u, cs, wedge), o in items:
                done += 1
                for par in range(2):
                    h = hp * 2 + par
                    nc.tensor.matmul(
                        cps[par][:, cs:QT],
                        va_sb[:, u, AUG * h:AUG * (h + 1)],
                        exps[par][:, o:o + (QT - cs)],
                        start=(done == 1), stop=(done == n_items),
                        skip_group_check=True)
            state[(hp, t)][1] = done
            if done == n_items:
                _finish_qt(hp, t, cps)
                del state[(hp, t)]

        recs_map = {}

        def _finish_qt(hp, t, cps):
            tq = slice(QT * t, QT * (t + 1))
            if hp not in recs_map:
                recs_map[hp] = (
                    precs.tile([65, NT * 2 * QT], f32r, tag="recs",
                               name=f"recs{hp}"),
                    precs.tile([1, NT * 2 * QT], f32r, tag="recs0",
                               name=f"recs0_{hp}"))
            recs, recs0 = recs_map[hp]
            for par in range(2):
                rc = slice((par * NT + t) * QT, (par * NT + t + 1) * QT)
                with nc.allow_low_precision(
                        reason="f32r is a bit-identical f32 alias"):
                    nc.vector.reciprocal(recs[64:65, rc],
                                         cps[par][HD:HD + 1, :])
                if par == 0:
                    nc.vector.tensor_copy(ctxT_sb[0:64, hp, tq],
                                          cps[par][0:HD, :])
                else:
                    st = sout.tile([64, QT], f32r, tag="st")
                    nc.vector.tensor_copy(st[:], cps[par][0:HD, :])
                    nc.sync.dma_start(ctxT_sb[64:128, hp, tq], st[:])
            if t == NT - 1:
                # normalize this head pair: broadcast 1/sum down partitions
                nc.sync.dma_start(recs0[:], recs[64:65, :])
                rb = sout.tile([128, GQ], f32r, tag="rb")
                for par in range(2):
                    rp = pout.tile([64, GQ], f32, tag="p1",
                                   name=f"rp{hp}_{par}")
                    nc.tensor.matmul(rp[:], ones_sb[0:1, 0:64],
                                     recs0[0:1, par * GQ:(par + 1) * GQ],
                                     start=True, stop=True)
                    if par == 0:
                        nc.vector.tensor_copy(rb[0:64, :], rp[:])
                    else:
                        st2 = sout.tile([64, GQ], f32r, tag="st")
                        nc.vector.tensor_copy(st2[:], rp[:])
                        nc.sync.dma_start(rb[64:128, :], st2[:])
                nc.vector.tensor_tensor(ctxT_sb[:, hp, :],
                                        ctxT_sb[:, hp, :], rb[:], ALU.mult)

        pipe_sps = {0: emit_scores(0)}
        pipe_exps = {}
        for i in range(len(flat)):
            if i + 1 < len(flat):
                pipe_sps[i + 1] = emit_scores(i + 1)
            pipe_exps[i] = emit_exp(i, pipe_sps.pop(i))
            if i - 1 >= 0:
                emit_ctx(i - 1, pipe_exps.pop(i - 1))
        last = len(flat) - 1
        emit_ctx(last, pipe_exps.pop(last))

        # ---------- phase 4: output projection + bias ----------
        for m in range(GQ // 128):
            for n in range(D // 512):
                ps = pout.tile([128, 512], f32, tag="p1")
                for k in range(D // 128):
                    nc.tensor.matmul(
                        ps[:], ctxT_sb[:, k, 128 * m:128 * (m + 1)],
                        Wo_sb[:, k, 512 * n:512 * (n + 1)],
                        start=(k == 0), stop=False)
                nc.tensor.matmul(ps[:], ones_sb[0:1, 0:128],
                                 bo_sb[0:1, 512 * n:512 * (n + 1)],
                                 start=False, stop=True)
                ob = sout.tile([128, 512], f32, tag="ob")
                nc.scalar.copy(ob[:], ps[:])
                nc.sync.dma_start(
                    out[128 * m:128 * (m + 1), 512 * n:512 * (n + 1)], ob[:])

    nc.compile()
    return nc


def _in_maps(x, offset, Wq, Wdkv, Wukv, Wo, bo):
    import ml_dtypes
    work = _worklist(offset)
    f32 = np.float32
    maps = []
    i32 = np.eye(32, dtype=ml_dtypes.bfloat16)
    common = {
        "Wq": np.ascontiguousarray(Wq, f32),
        "Wdkv": np.ascontiguousarray(Wdkv).astype(ml_dtypes.bfloat16),
        "Wukv": np.ascontiguousarray(Wukv, f32),
        "Wo": np.ascontiguousarray(Wo, f32),
        "bo": np.ascontiguousarray(bo, f32).reshape(1, D),
        "I32": i32,
        "Ones": np.ones((1, 130), f32),
    }
    for c in range(NCORES):
        b, g = c // 4, c % 4
        m = dict(common)
        m["xT"] = np.ascontiguousarray(x[b].T).astype(ml_dtypes.bfloat16)
        m["xqT"] = np.ascontiguousarray(x[b, g::4].T, f32)
        m["Twedge"] = _wedge_matrix(g, offset, work).astype(ml_dtypes.bfloat16)
        maps.append(m)
    return maps


def kernel(x, offset, Wq, Wdkv, Wukv, Wo, bo):
    from concourse.bass_utils import run_bass_kernel_spmd
    off = int(np.asarray(offset))
    if off not in _cache:
        _cache[off] = _build(off)
    nc = _cache[off]
    maps = _in_maps(np.asarray(x, np.float32), off, Wq, Wdkv, Wukv, Wo, bo)
    res = run_bass_kernel_spmd(nc, maps, list(range(NCORES)))
    outf = np.empty((B, S, D), np.float32)
    for c in range(NCORES):
        b, g = c // 4, c % 4
        outf[b, g::4, :] = res.results[c]["out"]
    return outf

